# revision 20
# baseline (speedup 1.0000x reference)
"""Trainium2 Bass kernel for nn_AbstractAttention (B=2, S=2048, D=1024, H=16, dh=64).

Sharding: 8 cores = 2 batch groups x 4 cores. Core i handles batch i//4 and
heads 4*(i%4)..+4 for QKV projection + causal attention; z is AllGathered
(fp16) within each 4-core group per query-quarter and every core then runs the
output projection for its own 256-column slice of d_model; the host
concatenates the 4 column slices per batch.

Structure (v2):
  - host pre-arranges x quarter-major ([qc][p][c][s]) and weights [p][c][e] so
    every DMA reads 4-8 KB contiguous per partition (8x fewer descriptors).
  - all 4 heads are processed inside one query-chunk loop as two head PAIRS.
    A pair's score matmuls are contraction-64 and land on the two 64x128 PE
    row-tiles (partitions 0:64 / 64:128), so the two heads' score blocks run
    CONCURRENTLY on the systolic array; PV/proj/out-proj (contraction-128)
    run in 128x128 mode, phase-grouped to limit tile-mode drains.
  - v_aug carries 64 ones-columns so the PV matmul broadcasts the softmax
    denominator into partitions 64:128 of zps; normalization is a single
    custom-DVE reciprocal_approx_fast + one multiply.
  - exp runs on 1024-wide paired tiles; diagonal-block causal masks are
    merged into one strided-AP multiply per unit.
  - per query-quarter, z of ALL 4 heads is staged as [128, 2*QC] and
    AllGathered in one collective (last quarter split in two); out-projection
    for that quarter's 4 row-blocks runs as interleaved filler work as soon
    as the gathered z lands, so only a small AG + 4 row-blocks trail the
    final attention.
  - a tiny dummy AllGather right at kernel start absorbs the one-time ~50us
    ncfw init under the projection phase.
"""
import os, sys, types

sys.path.insert(0, "/opt/trn_rl_repo")
import numpy as np

import concourse.bass as bass
import concourse.bacc as bacc
import concourse.tile as tile
from concourse import mybir
from concourse.bass_utils import run_bass_kernel_spmd

B, S, D, H, DH = 2, 2048, 1024, 16, 64
N_CORES = 8
HPC = 4            # heads per core
QC = 512           # query chunk width for score tiles
NQC = S // QC      # 4
KB = 128           # key block
NKB = S // KB      # 16
NDMC = D // 128    # 8 d_model chunks
DO = D // 4        # out-projection d_model columns per core
LEAD = 2           # score units in flight ahead of PV
F16 = mybir.dt.float16
F32 = mybir.dt.float32
RG = [[0, 1, 2, 3], [4, 5, 6, 7]]


def _install_ntff_hook():
    """Register the axon NTFF profiling hook missing from this image's antenv."""
    if "antenv.axon_hooks" in sys.modules:
        return
    try:
        from trn_agent_boot.trn_boot import _ntff_profile_via_ctypes

        hook = _ntff_profile_via_ctypes("/opt/axon/libaxon_pjrt.so")
        if hook is None:
            return
        import antenv  # noqa: F401

        mod = types.ModuleType("antenv.axon_hooks")
        mod.get_axon_ntff_profile_hook = lambda: hook
        sys.modules["antenv.axon_hooks"] = mod
    except Exception:
        pass


def build():
    nc = bacc.Bacc("TRN2", target_bir_lowering=False, debug=False, num_devices=N_CORES)
    # x: quarter-major [qc*128 + p, c*QC + s]; w: [p, c*(he) + e]
    xq = nc.dram_tensor("xq", [NQC * 128, NDMC * QC], F16, kind="ExternalInput")
    xk = nc.dram_tensor("xk", [NQC * 128, NDMC * QC], F16, kind="ExternalInput")
    xv = nc.dram_tensor("xv", [NQC * 128, NDMC * QC], F16, kind="ExternalInput")
    wq = nc.dram_tensor("wq", [128, NDMC * HPC * DH], F16, kind="ExternalInput")
    wk = nc.dram_tensor("wk", [128, NDMC * HPC * DH], F16, kind="ExternalInput")
    wv = nc.dram_tensor("wv", [128, NDMC * HPC * DH], F16, kind="ExternalInput")
    wo = nc.dram_tensor("wo", [128, NDMC * DO], F16, kind="ExternalInput")
    bq = nc.dram_tensor("bq", [HPC, DH], F32, kind="ExternalInput")
    bk = nc.dram_tensor("bk", [HPC, DH], F32, kind="ExternalInput")
    bv = nc.dram_tensor("bv", [HPC, DH], F32, kind="ExternalInput")
    bo = nc.dram_tensor("bo", [DO], F32, kind="ExternalInput")
    out = nc.dram_tensor("out", [S, DO], F32, kind="ExternalOutput")

    tri_dram = nc.inline_tensor(np.triu(np.ones((128, 128), np.float16)), name="tri_c")
    warm_src = nc.inline_tensor(np.ones((64, 16), np.float16), name="warm_c")
    salt = int(os.environ.get("KERNEL_SALT", "0"))
    salt_dram = (
        nc.inline_tensor(np.full((1, 16 * salt), 1.0, np.float32), name="salt_c")
        if salt
        else None
    )

    with tile.TileContext(nc) as tc:
        with (
            tc.tile_pool(name="consts", bufs=1) as consts,
            tc.tile_pool(name="persist", bufs=1) as persist,
            tc.tile_pool(name="xpool", bufs=2) as xpool,
            tc.tile_pool(name="ptp", bufs=8) as ptp,
            tc.tile_pool(name="recp", bufs=2) as recp,
            tc.tile_pool(name="obp", bufs=2) as obp,
            tc.tile_pool(name="psB", bufs=3, space="PSUM") as psB,
            tc.tile_pool(name="psC", bufs=2, space="PSUM") as psC,
            tc.tile_pool(name="dram", bufs=1, space="DRAM") as dram,
        ):
            # ---- gpsimd queue: warmup AG first (absorb ncfw init ASAP) ------
            warm_in = dram.tile([64, 16], F16, tag="warmin")
            warm_out = dram.tile([4, 64, 16], F16, tag="warmout")
            nc.gpsimd.dma_start(out=warm_in, in_=warm_src.ap())
            nc.gpsimd.collective_compute(
                "AllGather",
                mybir.AluOpType.bypass,
                replica_groups=RG,
                ins=[warm_in.opt()],
                outs=[warm_out.opt()],
            )
            wk_sb = consts.tile([128, NDMC, HPC * DH], F16, tag="wk")
            nc.gpsimd.dma_start(out=wk_sb, in_=wk.ap().rearrange("p (c e) -> p c e", c=NDMC))
            wq_sb = consts.tile([128, NDMC, HPC * DH], F16, tag="wq")
            nc.gpsimd.dma_start(out=wq_sb, in_=wq.ap().rearrange("p (c e) -> p c e", c=NDMC))
            tri = consts.tile([128, 128], F16, tag="tri")
            nc.gpsimd.dma_start(out=tri, in_=tri_dram.ap())
            bk_sb = consts.tile([128, 2], F32, tag="bk")
            bq_sb = consts.tile([128, 2], F32, tag="bq")
            for hp in range(2):
                nc.gpsimd.dma_start(
                    out=bk_sb[:, hp : hp + 1],
                    in_=bass.AP(tensor=bk.ap().tensor, offset=128 * hp, ap=[[1, 128], [1, 1]]),
                )
                nc.gpsimd.dma_start(
                    out=bq_sb[:, hp : hp + 1],
                    in_=bass.AP(tensor=bq.ap().tensor, offset=128 * hp, ap=[[1, 128], [1, 1]]),
                )
            bv_sb = consts.tile([128, HPC, DH], F32, tag="bv")
            nc.gpsimd.dma_start(
                out=bv_sb,
                in_=bass.AP(tensor=bv.ap().tensor, offset=0, ap=[[0, 128], [64, HPC], [1, DH]]),
            )
            wv_sb = consts.tile([128, NDMC, HPC * DH], F16, tag="wv")
            nc.gpsimd.dma_start(out=wv_sb, in_=wv.ap().rearrange("p (c e) -> p c e", c=NDMC))
            wo_sb = consts.tile([128, NDMC, DO], F16, tag="wo")
            nc.gpsimd.dma_start(out=wo_sb, in_=wo.ap().rearrange("p (c e) -> p c e", c=NDMC))
            bo_sb = consts.tile([128, DO], F32, tag="bo")
            nc.gpsimd.dma_start(
                out=bo_sb,
                in_=bass.AP(tensor=bo.ap().tensor, offset=0, ap=[[0, 128], [1, DO]]),
            )
            if salt_dram is not None:
                salt_sb = consts.tile([1, 16 * salt], F32, tag="salt")
                nc.gpsimd.dma_start(out=salt_sb, in_=salt_dram.ap())

            # ---- persistent activation tiles --------------------------------
            kT = persist.tile([128, 2, S], F16, tag="kT")   # [2 heads stacked, hp, pos]
            qTs = [
                persist.tile([128, 2, QC], F16, tag=f"qT{pc}", name=f"qT{pc}")
                for pc in range(NQC)
            ]
            # 64 ones-columns broadcast the softmax denominator into zps 64:128
            v_aug = persist.tile([128, NKB, HPC, 2 * DH], F16, tag="vaug")
            nc.vector.memset(v_aug[:, :, :, DH : 2 * DH], 1.0)
            # z staging per quarter: [2 heads stacked, hp, pos-in-quarter]
            zq_stage = [
                persist.tile([128, 2, QC], F16, tag=f"zq{pc}", name=f"zq{pc}")
                for pc in range(NQC)
            ]
            # per-quarter gathered z: avoids whole-tile false deps between
            # quarters' AG loads and out-projection reads
            zfull_q = [
                persist.tile([128, 8, QC], F16, tag=f"zf{pc}", name=f"zf{pc}")
                for pc in range(NQC)
            ]

            # ---- x loads: one contiguous 1 MB DMA per quarter ---------------
            def load_quarter(x_dram, pc, pfx, eng):
                t = xpool.tile([128, NDMC, QC], F16, tag=f"x{pfx}", name=f"x{pfx}{pc}")
                eng.dma_start(
                    out=t,
                    in_=x_dram.ap()[128 * pc : 128 * (pc + 1), :].rearrange(
                        "p (c s) -> p c s", c=NDMC
                    ),
                )
                return t

            # three queues load k/q/v quarters concurrently
            tk, tv, tq = [None] * NQC, [None] * NQC, [None] * NQC
            for pc in range(NQC):
                tk[pc] = load_quarter(xk, pc, "k", nc.sync)
                tq[pc] = load_quarter(xq, pc, "q", nc.sync)
                tv[pc] = load_quarter(xv, pc, "v", nc.scalar)

            # ---- projection thunks ------------------------------------------
            def proj_qk_thunk(dst, dst_sl, w_sb, b_sb, t, hp):
                def thunk():
                    pj_full = psB.tile([128, 1024], F32, tag="st")
                    pj = pj_full[:, 0:512]
                    for dmc in range(NDMC):
                        nc.tensor.matmul(
                            pj,
                            w_sb[:, dmc, 128 * hp : 128 * (hp + 1)],
                            t[:, dmc],
                            start=(dmc == 0),
                            stop=(dmc == NDMC - 1),
                        )
                    nc.vector.tensor_scalar_add(
                        dst[:, hp, dst_sl], pj, b_sb[:, hp : hp + 1]
                    )
                return thunk

            def proj_v_thunk(t, pc, pb4):
                def thunk():
                    pv_full = psB.tile([128, 1024], F32, tag="st")
                    pv = pv_full[:, 0 : HPC * DH]
                    for dmc in range(NDMC):
                        nc.tensor.matmul(
                            pv,
                            t[:, dmc, 128 * pb4 : 128 * (pb4 + 1)],
                            wv_sb[:, dmc],
                            start=(dmc == 0),
                            stop=(dmc == NDMC - 1),
                        )
                    kb = 4 * pc + pb4
                    nc.vector.tensor_add(
                        v_aug[:, kb, :, 0:DH],
                        pv.rearrange("p (h e) -> p h e", h=HPC),
                        bv_sb,
                    )
                return thunk

            def proj_thunks(pc):
                """[K hp0, Q hp0, V pb0, V pb1], [K hp1, Q hp1, V pb2, V pb3]"""
                sl = slice(QC * pc, QC * (pc + 1))
                f01 = [
                    proj_qk_thunk(kT, sl, wk_sb, bk_sb, tk[pc], 0),
                    proj_qk_thunk(qTs[pc], slice(0, QC), wq_sb, bq_sb, tq[pc], 0),
                    proj_v_thunk(tv[pc], pc, 0),
                    proj_v_thunk(tv[pc], pc, 1),
                ]
                f23 = [
                    proj_qk_thunk(kT, sl, wk_sb, bk_sb, tk[pc], 1),
                    proj_qk_thunk(qTs[pc], slice(0, QC), wq_sb, bq_sb, tq[pc], 1),
                    proj_v_thunk(tv[pc], pc, 2),
                    proj_v_thunk(tv[pc], pc, 3),
                ]
                return f01, f23

            # ---- out-projection ---------------------------------------------
            def outproj_qb_thunk(qb):
                def thunk():
                    po_full = psB.tile([128, 1024], F32, tag="st")
                    po = po_full[:, 0:DO]
                    pc, qo = qb // 4, qb % 4
                    for c in range(8):
                        nc.tensor.matmul(
                            po,
                            zfull_q[pc][:, c, 128 * qo : 128 * (qo + 1)],
                            wo_sb[:, c],
                            start=(c == 0),
                            stop=(c == 7),
                        )
                    ob = obp.tile([128, DO], F32, tag="ob")
                    nc.vector.tensor_add(ob, po, bo_sb)
                    nc.sync.dma_start(
                        out=out.ap()[128 * qb : 128 * (qb + 1), :], in_=ob
                    )
                return thunk

            # ---- attention: one head PAIR over one query chunk --------------
            def attention_pair(hp, qc, filler):
                """Heads 2*hp and 2*hp+1 over query chunk qc. Score matmuls of
                the two heads run on PE row-tiles (0,0)/(64,0) concurrently.
                filler: list of thunks emitting 128-mode PE work, popped one
                per PV cycle to fill PE time while ACT (exp) catches up."""
                nblk = 4 * qc + 4
                units = []
                for kb in range(0, 4 * qc, 2):
                    units.append([(kb, 0, 0, 512), (kb + 1, 512, 0, 512)])
                units.append([(4 * qc, 0, 0, 512), (4 * qc + 1, 512, 128, 384)])
                units.append([(4 * qc + 2, 0, 256, 256), (4 * qc + 3, 256, 384, 128)])
                nu = len(units)
                zps_ = [
                    psC.tile([128, QC], F32, tag="zps", name=f"zps{hp}{qc}{a}")
                    for a in range(2)
                ]
                pts = {}

                def emit_scores(ui):
                    unit = units[ui]
                    st = [
                        psB.tile([128, 1024], F32, tag="st", name=f"st{a}")
                        for a in range(2)
                    ]
                    for kb, co, off, w in unit:
                        for a in range(2):
                            m0 = 64 * a
                            nc.tensor.matmul(
                                st[a][:, co : co + w],
                                kT[m0 : m0 + 64, hp, 128 * kb : 128 * (kb + 1)],
                                qTs[qc][m0 : m0 + 64, hp, off:QC],
                                start=True,
                                stop=True,
                            )
                    tw = unit[-1][1] + unit[-1][3]
                    diag = unit[0][0] >= 4 * qc
                    for a in range(2):
                        pt = ptp.tile([128, 1024], F16, tag="pt")
                        nc.scalar.activation(
                            pt[:, 0:tw],
                            st[a][:, 0:tw],
                            mybir.ActivationFunctionType.Exp,
                            scale=0.125,
                        )
                        if diag:
                            # both 128-wide diagonal regions in one strided op
                            stride = unit[1][1]
                            view = pt[:, 0 : 2 * stride].rearrange(
                                "p (b c) -> p b c", b=2
                            )[:, :, 0:128]
                            tri_b = tri[:, :].unsqueeze(1).broadcast_to((128, 2, 128))
                            nc.vector.tensor_mul(view, view, tri_b)
                        pts[(a, ui)] = pt

                def emit_pvs(ui):
                    for a in range(2):
                        pt = pts.pop((a, ui))
                        h = 2 * hp + a
                        for kb, co, off, w in units[ui]:
                            nc.tensor.matmul(
                                zps_[a][:, off:QC],
                                v_aug[:, kb, h],
                                pt[:, co : co + w],
                                start=(kb == 0),
                                stop=(kb == nblk - 1),
                            )

                for ui in range(min(LEAD, nu)):
                    emit_scores(ui)
                for ui in range(nu):
                    emit_pvs(ui)
                    if filler:
                        filler.pop(0)()
                    if ui + LEAD < nu:
                        emit_scores(ui + LEAD)
                # normalize: z = zps[0:64] / zps[64:128] (64 identical denom rows)
                for a in range(2):
                    den = recp.tile([64, QC], F32, tag="den", name=f"den{a}")
                    nc.vector.tensor_copy(den, zps_[a][64:128, :])
                    rec = recp.tile([64, QC], F32, tag="rec", name=f"rec{a}")
                    nc.vector.reciprocal_approx_fast(rec, den)
                    nc.vector.tensor_mul(
                        zq_stage[qc][64 * a : 64 * (a + 1), hp, :],
                        zps_[a][0:64, :],
                        rec,
                    )
                while filler:
                    filler.pop(0)()

            # ---- per-quarter z AllGather ------------------------------------
            def ag_quarter(pc, half=None):
                """AllGather z for quarter pc. half=None: both head pairs in
                one collective; half=0/1: only pair hp=half (used for qc3)."""
                if half is None:
                    zin = dram.tile([128, 2, QC], F16, tag=f"zi{pc}", name=f"zi{pc}")
                    zout = dram.tile(
                        [4, 128, 2, QC], F16, tag=f"zo{pc}", name=f"zo{pc}"
                    )
                    nc.sync.dma_start(out=zin, in_=zq_stage[pc][:, :, :])
                else:
                    zin = dram.tile(
                        [128, QC], F16, tag=f"zi{pc}h{half}", name=f"zi{pc}h{half}"
                    )
                    zout = dram.tile(
                        [4, 128, QC], F16, tag=f"zo{pc}h{half}", name=f"zo{pc}h{half}"
                    )
                    nc.sync.dma_start(out=zin, in_=zq_stage[pc][:, half, :])
                nc.gpsimd.collective_compute(
                    "AllGather",
                    mybir.AluOpType.bypass,
                    replica_groups=RG,
                    ins=[zin.opt()],
                    outs=[zout.opt()],
                )
                # gathered z -> SBUF on the gpsimd queue: the AG-done semaphore
                # wait must NOT sit on the scalar/vector queues where it would
                # block later exp/DVE work; gpsimd only carries AG triggers,
                # which serialize on the CC anyway.
                if half is None:
                    nc.gpsimd.dma_start(
                        out=zfull_q[pc][:, :, :].rearrange(
                            "p (j h) s -> p j h s", j=4
                        ),
                        in_=zout[:, :, :, :].transpose([1, 0, 2, 3]),
                    )
                else:
                    for j in range(4):
                        nc.gpsimd.dma_start(
                            out=zfull_q[pc][:, 2 * j + half, :], in_=zout[j]
                        )

            # ---- main schedule ----------------------------------------------
            # qc0: only K/Q hp0 + all V upfront; K/Q hp1 rides inside pair01
            p01, p23 = proj_thunks(0)
            for th in [p01[0], p01[1], p01[2], p01[3], p23[2], p23[3]]:
                th()
            carry = [p23[0], p23[1]]
            for qc in range(NQC):
                if qc < NQC - 1:
                    f01, f23 = proj_thunks(qc + 1)
                else:
                    f01, f23 = [], []
                f01 = carry + f01
                carry = []
                if qc == 2:
                    # quarter-0 AG landed during qc1; out-proj rides as filler
                    f01 += [outproj_qb_thunk(qb) for qb in range(0, 2)]
                    f23 += [outproj_qb_thunk(qb) for qb in range(2, 4)]
                if qc == 3:
                    # only quarter-1 (whose AG completed long ago) as filler;
                    # quarters 2/3 out-proj goes after all attention so PE
                    # never stalls on a late AllGather mid-attention
                    f01 += [outproj_qb_thunk(qb) for qb in range(4, 6)]
                    f23 += [outproj_qb_thunk(qb) for qb in range(6, 8)]
                attention_pair(0, qc, f01)
                if qc == NQC - 1:
                    ag_quarter(qc, half=0)
                attention_pair(1, qc, f23)
                if qc < NQC - 1:
                    ag_quarter(qc)
                else:
                    ag_quarter(qc, half=1)
            # tail: quarters 2 and 3 out-projection
            for qb in range(8, 16):
                outproj_qb_thunk(qb)()

    nc.finalize()
    return nc


_CACHE = {}


def kernel(**inputs):
    _install_ntff_hook()
    nc = _CACHE.get("nc")
    if nc is None:
        nc = build()
        _CACHE["nc"] = nc

    f16 = np.float16
    xs = {k: np.asarray(inputs[k], np.float32) for k in ("query_input", "key_input", "value_input")}
    W = {k: np.asarray(inputs[k], np.float32) for k in ("W_Q", "W_K", "W_V", "W_O")}
    b = {k: np.asarray(inputs[k], np.float32) for k in ("b_Q", "b_K", "b_V", "b_O")}
    # pre-arrange activations quarter-major [qc, p, c, s] so device DMAs are
    # fully contiguous per partition
    xT16 = {}
    for k, v in xs.items():
        xT16[k] = []
        for g in range(B):
            xT = v[g].T  # [D, S]
            arr = (
                xT.reshape(NDMC, 128, NQC, QC)
                .transpose(2, 1, 0, 3)
                .reshape(NQC * 128, NDMC * QC)
            )
            xT16[k].append(np.ascontiguousarray(arr).astype(f16))
    # projection weights [p, c, he] per head group
    wd = {}
    for k in ("W_Q", "W_K", "W_V"):
        wd[k] = []
        for r in range(4):
            wde = W[k][4 * r : 4 * (r + 1)].transpose(1, 0, 2).reshape(D, HPC * DH)
            arr = (
                wde.reshape(NDMC, 128, HPC * DH)
                .transpose(1, 0, 2)
                .reshape(128, NDMC * HPC * DH)
            )
            wd[k].append(np.ascontiguousarray(arr).astype(f16))
    wo_full = W["W_O"].reshape(H * DH, D)
    wo_slices = []
    for r in range(4):
        ws = wo_full[:, DO * r : DO * (r + 1)]  # [1024, 256]
        arr = ws.reshape(NDMC, 128, DO).transpose(1, 0, 2).reshape(128, NDMC * DO)
        wo_slices.append(np.ascontiguousarray(arr).astype(f16))
    bo_slices = [np.ascontiguousarray(b["b_O"][DO * r : DO * (r + 1)]) for r in range(4)]

    in_maps = []
    for i in range(N_CORES):
        g, r = i // 4, i % 4
        in_maps.append(
            {
                "xq": xT16["query_input"][g],
                "xk": xT16["key_input"][g],
                "xv": xT16["value_input"][g],
                "wq": wd["W_Q"][r],
                "wk": wd["W_K"][r],
                "wv": wd["W_V"][r],
                "wo": wo_slices[r],
                "bq": np.ascontiguousarray(b["b_Q"][4 * r : 4 * (r + 1)]),
                "bk": np.ascontiguousarray(b["b_K"][4 * r : 4 * (r + 1)]),
                "bv": np.ascontiguousarray(b["b_V"][4 * r : 4 * (r + 1)]),
                "bo": bo_slices[r],
            }
        )

    res = run_bass_kernel_spmd(nc, in_maps, core_ids=list(range(N_CORES)))
    if os.environ.get("KERNEL_PRINT_EXEC"):
        print(f"HW exec time: {res.exec_time_ns} ns")
    outs = []
    for g in range(B):
        outs.append(
            np.concatenate([res.results[4 * g + r]["out"] for r in range(4)], axis=1)
        )
    return np.stack(outs, axis=0).astype(np.float32)


# revision 23
# speedup vs baseline: 1.0167x; 1.0167x over previous
"""Trainium2 Bass kernel for nn_AbstractAttention (B=2, S=2048, D=1024, H=16, dh=64).

Sharding: 8 cores = 2 batch groups x 4 cores. Core i handles batch i//4 and
heads 4*(i%4)..+4 for QKV projection + causal attention; z is AllGathered
(fp16) within each 4-core group per query-quarter and every core then runs the
output projection for its own 256-column slice of d_model; the host
concatenates the 4 column slices per batch.

Structure (v2):
  - host pre-arranges x quarter-major ([qc][p][c][s]) and weights [p][c][e] so
    every DMA reads 4-8 KB contiguous per partition (8x fewer descriptors).
  - all 4 heads are processed inside one query-chunk loop as two head PAIRS.
    A pair's score matmuls are contraction-64 and land on the two 64x128 PE
    row-tiles (partitions 0:64 / 64:128), so the two heads' score blocks run
    CONCURRENTLY on the systolic array; PV/proj/out-proj (contraction-128)
    run in 128x128 mode, phase-grouped to limit tile-mode drains.
  - v_aug carries 64 ones-columns so the PV matmul broadcasts the softmax
    denominator into partitions 64:128 of zps; normalization is a single
    custom-DVE reciprocal_approx_fast + one multiply.
  - exp runs on 1024-wide paired tiles; diagonal-block causal masks are
    merged into one strided-AP multiply per unit.
  - per query-quarter, z of ALL 4 heads is staged as [128, 2*QC] and
    AllGathered in one collective (last quarter split in two); out-projection
    for that quarter's 4 row-blocks runs as interleaved filler work as soon
    as the gathered z lands, so only a small AG + 4 row-blocks trail the
    final attention.
  - a tiny dummy AllGather right at kernel start absorbs the one-time ~50us
    ncfw init under the projection phase.
"""
import os, sys, types

sys.path.insert(0, "/opt/trn_rl_repo")
import numpy as np

import concourse.bass as bass
import concourse.bacc as bacc
import concourse.tile as tile
from concourse import mybir
from concourse.bass_utils import run_bass_kernel_spmd

B, S, D, H, DH = 2, 2048, 1024, 16, 64
N_CORES = 8
HPC = 4            # heads per core
QC = 512           # query chunk width for score tiles
NQC = S // QC      # 4
KB = 128           # key block
NKB = S // KB      # 16
NDMC = D // 128    # 8 d_model chunks
DO = D // 4        # out-projection d_model columns per core
LEAD = 2           # score units in flight ahead of PV
F16 = mybir.dt.float16
F32 = mybir.dt.float32
RG = [[0, 1, 2, 3], [4, 5, 6, 7]]


def _install_ntff_hook():
    """Register the axon NTFF profiling hook missing from this image's antenv."""
    if "antenv.axon_hooks" in sys.modules:
        return
    try:
        from trn_agent_boot.trn_boot import _ntff_profile_via_ctypes

        hook = _ntff_profile_via_ctypes("/opt/axon/libaxon_pjrt.so")
        if hook is None:
            return
        import antenv  # noqa: F401

        mod = types.ModuleType("antenv.axon_hooks")
        mod.get_axon_ntff_profile_hook = lambda: hook
        sys.modules["antenv.axon_hooks"] = mod
    except Exception:
        pass


def build():
    nc = bacc.Bacc("TRN2", target_bir_lowering=False, debug=False, num_devices=N_CORES)
    # x: quarter-major [qc*128 + p, c*QC + s]; w: [p, c*(he) + e]
    xq = nc.dram_tensor("xq", [NQC * 128, NDMC * QC], F16, kind="ExternalInput")
    xk = nc.dram_tensor("xk", [NQC * 128, NDMC * QC], F16, kind="ExternalInput")
    xv = nc.dram_tensor("xv", [NQC * 128, NDMC * QC], F16, kind="ExternalInput")
    wq = nc.dram_tensor("wq", [128, NDMC * HPC * DH], F16, kind="ExternalInput")
    wk = nc.dram_tensor("wk", [128, NDMC * HPC * DH], F16, kind="ExternalInput")
    wv = nc.dram_tensor("wv", [128, NDMC * HPC * DH], F16, kind="ExternalInput")
    wo = nc.dram_tensor("wo", [128, NDMC * DO], F16, kind="ExternalInput")
    bq = nc.dram_tensor("bq", [HPC, DH], F32, kind="ExternalInput")
    bk = nc.dram_tensor("bk", [HPC, DH], F32, kind="ExternalInput")
    bv = nc.dram_tensor("bv", [HPC, DH], F32, kind="ExternalInput")
    bo = nc.dram_tensor("bo", [DO], F32, kind="ExternalInput")
    out = nc.dram_tensor("out", [S, DO], F32, kind="ExternalOutput")

    tri_dram = nc.inline_tensor(np.triu(np.ones((128, 128), np.float16)), name="tri_c")
    warm_src = nc.inline_tensor(np.ones((64, 16), np.float16), name="warm_c")
    salt = int(os.environ.get("KERNEL_SALT", "0"))
    salt_dram = (
        nc.inline_tensor(np.full((1, 16 * salt), 1.0, np.float32), name="salt_c")
        if salt
        else None
    )

    with tile.TileContext(nc) as tc:
        with (
            tc.tile_pool(name="consts", bufs=1) as consts,
            tc.tile_pool(name="persist", bufs=1) as persist,
            tc.tile_pool(name="xpool", bufs=3) as xpool,
            tc.tile_pool(name="ptp", bufs=8) as ptp,
            tc.tile_pool(name="recp", bufs=2) as recp,
            tc.tile_pool(name="obp", bufs=2) as obp,
            tc.tile_pool(name="psB", bufs=3, space="PSUM") as psB,
            tc.tile_pool(name="psC", bufs=2, space="PSUM") as psC,
            tc.tile_pool(name="dram", bufs=1, space="DRAM") as dram,
        ):
            # ---- gpsimd queue: warmup AG first (absorb ncfw init ASAP) ------
            warm_in = dram.tile([64, 16], F16, tag="warmin")
            warm_out = dram.tile([4, 64, 16], F16, tag="warmout")
            nc.gpsimd.dma_start(out=warm_in, in_=warm_src.ap())
            nc.gpsimd.collective_compute(
                "AllGather",
                mybir.AluOpType.bypass,
                replica_groups=RG,
                ins=[warm_in.opt()],
                outs=[warm_out.opt()],
            )
            # K/Q weights ride the sync HWDGE queue ahead of the x loads so
            # the first projection matmul can start ~12us in
            wk_sb = consts.tile([128, NDMC, HPC * DH], F16, tag="wk")
            nc.sync.dma_start(out=wk_sb, in_=wk.ap().rearrange("p (c e) -> p c e", c=NDMC))
            wq_sb = consts.tile([128, NDMC, HPC * DH], F16, tag="wq")
            nc.sync.dma_start(out=wq_sb, in_=wq.ap().rearrange("p (c e) -> p c e", c=NDMC))
            tri = consts.tile([128, 128], F16, tag="tri")
            nc.gpsimd.dma_start(out=tri, in_=tri_dram.ap())
            bk_sb = consts.tile([128, 2], F32, tag="bk")
            bq_sb = consts.tile([128, 2], F32, tag="bq")
            for hp in range(2):
                nc.gpsimd.dma_start(
                    out=bk_sb[:, hp : hp + 1],
                    in_=bass.AP(tensor=bk.ap().tensor, offset=128 * hp, ap=[[1, 128], [1, 1]]),
                )
                nc.gpsimd.dma_start(
                    out=bq_sb[:, hp : hp + 1],
                    in_=bass.AP(tensor=bq.ap().tensor, offset=128 * hp, ap=[[1, 128], [1, 1]]),
                )
            bv_sb = consts.tile([128, HPC, DH], F32, tag="bv")
            nc.gpsimd.dma_start(
                out=bv_sb,
                in_=bass.AP(tensor=bv.ap().tensor, offset=0, ap=[[0, 128], [64, HPC], [1, DH]]),
            )
            wv_sb = consts.tile([128, NDMC, HPC * DH], F16, tag="wv")
            nc.gpsimd.dma_start(out=wv_sb, in_=wv.ap().rearrange("p (c e) -> p c e", c=NDMC))
            wo_sb = consts.tile([128, NDMC, DO], F16, tag="wo")
            nc.gpsimd.dma_start(out=wo_sb, in_=wo.ap().rearrange("p (c e) -> p c e", c=NDMC))
            bo_sb = consts.tile([128, DO], F32, tag="bo")
            nc.gpsimd.dma_start(
                out=bo_sb,
                in_=bass.AP(tensor=bo.ap().tensor, offset=0, ap=[[0, 128], [1, DO]]),
            )
            if salt_dram is not None:
                salt_sb = consts.tile([1, 16 * salt], F32, tag="salt")
                nc.gpsimd.dma_start(out=salt_sb, in_=salt_dram.ap())

            # ---- persistent activation tiles --------------------------------
            kT = persist.tile([128, 2, S], F16, tag="kT")   # [2 heads stacked, hp, pos]
            qTs = [
                persist.tile([128, 2, QC], F16, tag=f"qT{pc}", name=f"qT{pc}")
                for pc in range(NQC)
            ]
            # 64 ones-columns broadcast the softmax denominator into zps 64:128
            v_aug = persist.tile([128, NKB, HPC, 2 * DH], F16, tag="vaug")
            nc.vector.memset(v_aug[:, :, :, DH : 2 * DH], 1.0)
            # z staging per quarter: [2 heads stacked, hp, pos-in-quarter]
            zq_stage = [
                persist.tile([128, 2, QC], F16, tag=f"zq{pc}", name=f"zq{pc}")
                for pc in range(NQC)
            ]
            # per-quarter gathered z: avoids whole-tile false deps between
            # quarters' AG loads and out-projection reads
            zfull_q = [
                persist.tile([128, 8, QC], F16, tag=f"zf{pc}", name=f"zf{pc}")
                for pc in range(NQC)
            ]

            # ---- x loads: one contiguous 1 MB DMA per quarter ---------------
            def load_quarter(x_dram, pc, pfx, eng):
                t = xpool.tile([128, NDMC, QC], F16, tag=f"x{pfx}", name=f"x{pfx}{pc}")
                eng.dma_start(
                    out=t,
                    in_=x_dram.ap()[128 * pc : 128 * (pc + 1), :].rearrange(
                        "p (c s) -> p c s", c=NDMC
                    ),
                )
                return t

            # all x on the sync queue: the scalar (ACT) queue must stay free
            # of DMA dispatches — a WAR-gated dispatch there blocks every exp
            # behind it
            tk, tv, tq = [None] * NQC, [None] * NQC, [None] * NQC
            for pc in range(NQC):
                tk[pc] = load_quarter(xk, pc, "k", nc.sync)
                tq[pc] = load_quarter(xq, pc, "q", nc.sync)
                tv[pc] = load_quarter(xv, pc, "v", nc.sync)

            # ---- projection thunks ------------------------------------------
            def proj_qk_thunk(dst, dst_sl, w_sb, b_sb, t, hp):
                def thunk():
                    pj_full = psB.tile([128, 1024], F32, tag="st")
                    pj = pj_full[:, 0:512]
                    for dmc in range(NDMC):
                        nc.tensor.matmul(
                            pj,
                            w_sb[:, dmc, 128 * hp : 128 * (hp + 1)],
                            t[:, dmc],
                            start=(dmc == 0),
                            stop=(dmc == NDMC - 1),
                        )
                    nc.vector.tensor_scalar_add(
                        dst[:, hp, dst_sl], pj, b_sb[:, hp : hp + 1]
                    )
                return thunk

            def proj_v_thunk(t, pc, pb4):
                def thunk():
                    pv_full = psB.tile([128, 1024], F32, tag="st")
                    pv = pv_full[:, 0 : HPC * DH]
                    for dmc in range(NDMC):
                        nc.tensor.matmul(
                            pv,
                            t[:, dmc, 128 * pb4 : 128 * (pb4 + 1)],
                            wv_sb[:, dmc],
                            start=(dmc == 0),
                            stop=(dmc == NDMC - 1),
                        )
                    kb = 4 * pc + pb4
                    nc.vector.tensor_add(
                        v_aug[:, kb, :, 0:DH],
                        pv.rearrange("p (h e) -> p h e", h=HPC),
                        bv_sb,
                    )
                return thunk

            def proj_thunks(pc):
                """[K hp0, Q hp0, V pb0, V pb1], [K hp1, Q hp1, V pb2, V pb3]"""
                sl = slice(QC * pc, QC * (pc + 1))
                f01 = [
                    proj_qk_thunk(kT, sl, wk_sb, bk_sb, tk[pc], 0),
                    proj_qk_thunk(qTs[pc], slice(0, QC), wq_sb, bq_sb, tq[pc], 0),
                    proj_v_thunk(tv[pc], pc, 0),
                    proj_v_thunk(tv[pc], pc, 1),
                ]
                f23 = [
                    proj_qk_thunk(kT, sl, wk_sb, bk_sb, tk[pc], 1),
                    proj_qk_thunk(qTs[pc], slice(0, QC), wq_sb, bq_sb, tq[pc], 1),
                    proj_v_thunk(tv[pc], pc, 2),
                    proj_v_thunk(tv[pc], pc, 3),
                ]
                return f01, f23

            # ---- out-projection ---------------------------------------------
            def outproj_qb_thunk(qb):
                def thunk():
                    po_full = psB.tile([128, 1024], F32, tag="st")
                    po = po_full[:, 0:DO]
                    pc, qo = qb // 4, qb % 4
                    for c in range(8):
                        nc.tensor.matmul(
                            po,
                            zfull_q[pc][:, c, 128 * qo : 128 * (qo + 1)],
                            wo_sb[:, c],
                            start=(c == 0),
                            stop=(c == 7),
                        )
                    ob = obp.tile([128, DO], F32, tag="ob")
                    nc.vector.tensor_add(ob, po, bo_sb)
                    nc.sync.dma_start(
                        out=out.ap()[128 * qb : 128 * (qb + 1), :], in_=ob
                    )
                return thunk

            # ---- attention: one head PAIR over one query chunk --------------
            def attention_pair(hp, qc, filler):
                """Heads 2*hp and 2*hp+1 over query chunk qc. Score matmuls of
                the two heads run on PE row-tiles (0,0)/(64,0) concurrently.
                filler: list of thunks emitting 128-mode PE work, popped one
                per PV cycle to fill PE time while ACT (exp) catches up."""
                nblk = 4 * qc + 4
                units = []
                for kb in range(0, 4 * qc, 2):
                    units.append([(kb, 0, 0, 512), (kb + 1, 512, 0, 512)])
                units.append([(4 * qc, 0, 0, 512), (4 * qc + 1, 512, 128, 384)])
                units.append([(4 * qc + 2, 0, 256, 256), (4 * qc + 3, 256, 384, 128)])
                nu = len(units)
                zps_ = [
                    psC.tile([128, QC], F32, tag="zps", name=f"zps{hp}{qc}{a}")
                    for a in range(2)
                ]
                pts = {}

                def emit_scores(ui):
                    unit = units[ui]
                    st = [
                        psB.tile([128, 1024], F32, tag="st", name=f"st{a}")
                        for a in range(2)
                    ]
                    for kb, co, off, w in unit:
                        for a in range(2):
                            m0 = 64 * a
                            nc.tensor.matmul(
                                st[a][:, co : co + w],
                                kT[m0 : m0 + 64, hp, 128 * kb : 128 * (kb + 1)],
                                qTs[qc][m0 : m0 + 64, hp, off:QC],
                                start=True,
                                stop=True,
                            )
                    tw = unit[-1][1] + unit[-1][3]
                    diag = unit[0][0] >= 4 * qc
                    for a in range(2):
                        pt = ptp.tile([128, 1024], F16, tag="pt")
                        nc.scalar.activation(
                            pt[:, 0:tw],
                            st[a][:, 0:tw],
                            mybir.ActivationFunctionType.Exp,
                            scale=0.125,
                        )
                        if diag:
                            # both 128-wide diagonal regions in one strided op
                            stride = unit[1][1]
                            view = pt[:, 0 : 2 * stride].rearrange(
                                "p (b c) -> p b c", b=2
                            )[:, :, 0:128]
                            tri_b = tri[:, :].unsqueeze(1).broadcast_to((128, 2, 128))
                            nc.vector.tensor_mul(view, view, tri_b)
                        pts[(a, ui)] = pt

                def emit_pvs(ui):
                    for a in range(2):
                        pt = pts.pop((a, ui))
                        h = 2 * hp + a
                        for kb, co, off, w in units[ui]:
                            nc.tensor.matmul(
                                zps_[a][:, off:QC],
                                v_aug[:, kb, h],
                                pt[:, co : co + w],
                                start=(kb == 0),
                                stop=(kb == nblk - 1),
                            )

                for ui in range(min(LEAD, nu)):
                    emit_scores(ui)
                for ui in range(nu):
                    emit_pvs(ui)
                    if filler:
                        filler.pop(0)()
                    if ui + LEAD < nu:
                        emit_scores(ui + LEAD)
                # normalize: z = zps[0:64] / zps[64:128] (64 identical denom rows)
                for a in range(2):
                    den = recp.tile([64, QC], F32, tag="den", name=f"den{a}")
                    nc.vector.tensor_copy(den, zps_[a][64:128, :])
                    rec = recp.tile([64, QC], F32, tag="rec", name=f"rec{a}")
                    nc.vector.reciprocal_approx_fast(rec, den)
                    nc.vector.tensor_mul(
                        zq_stage[qc][64 * a : 64 * (a + 1), hp, :],
                        zps_[a][0:64, :],
                        rec,
                    )
                while filler:
                    filler.pop(0)()

            # ---- per-quarter z AllGather ------------------------------------
            def ag_quarter(pc, half=None):
                """AllGather z for quarter pc. half=None: both head pairs in
                one collective; half=0/1: only pair hp=half (used for qc3)."""
                if half is None:
                    zin = dram.tile([128, 2, QC], F16, tag=f"zi{pc}", name=f"zi{pc}")
                    zout = dram.tile(
                        [4, 128, 2, QC], F16, tag=f"zo{pc}", name=f"zo{pc}"
                    )
                    nc.sync.dma_start(out=zin, in_=zq_stage[pc][:, :, :])
                else:
                    zin = dram.tile(
                        [128, QC], F16, tag=f"zi{pc}h{half}", name=f"zi{pc}h{half}"
                    )
                    zout = dram.tile(
                        [4, 128, QC], F16, tag=f"zo{pc}h{half}", name=f"zo{pc}h{half}"
                    )
                    nc.sync.dma_start(out=zin, in_=zq_stage[pc][:, half, :])
                nc.gpsimd.collective_compute(
                    "AllGather",
                    mybir.AluOpType.bypass,
                    replica_groups=RG,
                    ins=[zin.opt()],
                    outs=[zout.opt()],
                )
                # gathered z -> SBUF on the gpsimd queue: the AG-done semaphore
                # wait must NOT sit on the scalar/vector queues where it would
                # block later exp/DVE work; gpsimd only carries AG triggers,
                # which serialize on the CC anyway.
                if half is None:
                    nc.gpsimd.dma_start(
                        out=zfull_q[pc][:, :, :].rearrange(
                            "p (j h) s -> p j h s", j=4
                        ),
                        in_=zout[:, :, :, :].transpose([1, 0, 2, 3]),
                    )
                else:
                    for j in range(4):
                        nc.gpsimd.dma_start(
                            out=zfull_q[pc][:, 2 * j + half, :], in_=zout[j]
                        )

            # ---- main schedule ----------------------------------------------
            # qc0: only K/Q hp0 + all V upfront; K/Q hp1 rides inside pair01
            p01, p23 = proj_thunks(0)
            for th in [p01[0], p01[1], p01[2], p01[3], p23[2], p23[3]]:
                th()
            carry = [p23[0], p23[1]]
            for qc in range(NQC):
                if qc < NQC - 1:
                    f01, f23 = proj_thunks(qc + 1)
                else:
                    f01, f23 = [], []
                f01 = carry + f01
                carry = []
                if qc == 2:
                    # quarter-0 AG landed during qc1; out-proj rides as filler
                    f01 += [outproj_qb_thunk(qb) for qb in range(0, 2)]
                    f23 += [outproj_qb_thunk(qb) for qb in range(2, 4)]
                if qc == 3:
                    # only quarter-1 (whose AG completed long ago) as filler;
                    # quarters 2/3 out-proj goes after all attention so PE
                    # never stalls on a late AllGather mid-attention
                    f01 += [outproj_qb_thunk(qb) for qb in range(4, 6)]
                    f23 += [outproj_qb_thunk(qb) for qb in range(6, 8)]
                attention_pair(0, qc, f01)
                if qc == NQC - 1:
                    ag_quarter(qc, half=0)
                attention_pair(1, qc, f23)
                if qc < NQC - 1:
                    ag_quarter(qc)
                else:
                    ag_quarter(qc, half=1)
            # tail: quarters 2 and 3 out-projection
            for qb in range(8, 16):
                outproj_qb_thunk(qb)()

    nc.finalize()
    return nc


_CACHE = {}


def kernel(**inputs):
    _install_ntff_hook()
    nc = _CACHE.get("nc")
    if nc is None:
        nc = build()
        _CACHE["nc"] = nc

    f16 = np.float16
    xs = {k: np.asarray(inputs[k], np.float32) for k in ("query_input", "key_input", "value_input")}
    W = {k: np.asarray(inputs[k], np.float32) for k in ("W_Q", "W_K", "W_V", "W_O")}
    b = {k: np.asarray(inputs[k], np.float32) for k in ("b_Q", "b_K", "b_V", "b_O")}
    # pre-arrange activations quarter-major [qc, p, c, s] so device DMAs are
    # fully contiguous per partition
    xT16 = {}
    for k, v in xs.items():
        xT16[k] = []
        for g in range(B):
            xT = v[g].T  # [D, S]
            arr = (
                xT.reshape(NDMC, 128, NQC, QC)
                .transpose(2, 1, 0, 3)
                .reshape(NQC * 128, NDMC * QC)
            )
            xT16[k].append(np.ascontiguousarray(arr).astype(f16))
    # projection weights [p, c, he] per head group
    wd = {}
    for k in ("W_Q", "W_K", "W_V"):
        wd[k] = []
        for r in range(4):
            wde = W[k][4 * r : 4 * (r + 1)].transpose(1, 0, 2).reshape(D, HPC * DH)
            arr = (
                wde.reshape(NDMC, 128, HPC * DH)
                .transpose(1, 0, 2)
                .reshape(128, NDMC * HPC * DH)
            )
            wd[k].append(np.ascontiguousarray(arr).astype(f16))
    wo_full = W["W_O"].reshape(H * DH, D)
    wo_slices = []
    for r in range(4):
        ws = wo_full[:, DO * r : DO * (r + 1)]  # [1024, 256]
        arr = ws.reshape(NDMC, 128, DO).transpose(1, 0, 2).reshape(128, NDMC * DO)
        wo_slices.append(np.ascontiguousarray(arr).astype(f16))
    bo_slices = [np.ascontiguousarray(b["b_O"][DO * r : DO * (r + 1)]) for r in range(4)]

    in_maps = []
    for i in range(N_CORES):
        g, r = i // 4, i % 4
        in_maps.append(
            {
                "xq": xT16["query_input"][g],
                "xk": xT16["key_input"][g],
                "xv": xT16["value_input"][g],
                "wq": wd["W_Q"][r],
                "wk": wd["W_K"][r],
                "wv": wd["W_V"][r],
                "wo": wo_slices[r],
                "bq": np.ascontiguousarray(b["b_Q"][4 * r : 4 * (r + 1)]),
                "bk": np.ascontiguousarray(b["b_K"][4 * r : 4 * (r + 1)]),
                "bv": np.ascontiguousarray(b["b_V"][4 * r : 4 * (r + 1)]),
                "bo": bo_slices[r],
            }
        )

    res = run_bass_kernel_spmd(nc, in_maps, core_ids=list(range(N_CORES)))
    if os.environ.get("KERNEL_PRINT_EXEC"):
        print(f"HW exec time: {res.exec_time_ns} ns")
    outs = []
    for g in range(B):
        outs.append(
            np.concatenate([res.results[4 * g + r]["out"] for r in range(4)], axis=1)
        )
    return np.stack(outs, axis=0).astype(np.float32)
